# revision 37
# baseline (speedup 1.0000x reference)
"""ColBERT MaxSim loss kernel for Trainium2 (8 NeuronCores, SPMD).

Sharding: documents across the 8 cores (32 docs each); queries replicated.
Host prep ships embeddings pre-transposed ([H, tok]) in bf16, so the device
does no input transposes and reads half the bytes. Per core, per pass:

  1. Streaming driver: doc-embedding DMA chunks, projection groups, deposits
     and sim/reduce units are interleaved in emission order (the five engine
     queues are in-order, so emission order IS the schedule).
  2. Projection: bf16 matmuls over 6 H-ktiles -> PSUM f32 [tok, 64] in
     4-block groups; norms via Act square+accum -> Act sqrt (batched) -> DVE
     reciprocal -> one DVE broadcast tensor_tensor scale -> bf16 [tok, 64];
     PE-transpose to [64, tok]; DVE-deposit into the token-major dt_/qt_
     operand buffers; a DVE tensor_tensor builds pair-diff columns
     dd_ = d_even - d_odd per deposit.
  3. MaxSim reduce units (one per 4-doc chunk x 128-query-token block), in a
     tuned mix of two flavors:
       'D': sim matmul [128, 360] -> DVE reduce_max over Ld straight from
            PSUM (DVE is the only engine that can max; GPSIMD has no PSUM
            port and its tensor ops don't compile on public neuronxcc).
       'R': relu-pairmax, max(a,b) = relu(a-b) + b: PE matmuls
            s_diff = q*dd and s_odd = q*d_odd (stride-2 rhs view) into two
            single-bank PSUM tiles; Act relus s_diff to SBUF bf16; PE
            accumulates it onto s_odd (identity matmul, PSUM accumulate);
            DVE max-reduces the half-width [128, 4, 90] tile. This splits
            the reduce across PE/Act/DVE instead of saturating DVE.
  4. Lq-sum via a block-diagonal ones matmul; [32 x 32] score block to HBM.

The repeat loop (used by the timing harness) emits two passes per hardware
loop body with per-slot doct/qt/maxsim buffers so the next pass's input DMA
prefetches under the current pass's compute tail.

Host concatenates the 8 blocks into [32, 256] and finishes the (tiny) cross
entropy in f64. bf16 end-to-end loss rel err vs the f32 reference: ~1e-4
(gate is 2e-2); min token norm is ~5 so the reference's 1e-12 eps guard is
unreachable and skipped. Measured: 85.9us HW exec vs 292us for the staged
baseline kernel (same timing method), ~3.4x.
"""

import sys

import numpy as np

try:
    import concourse.bass as bass
except ImportError:  # pragma: no cover - fallback for bare environments
    sys.path.insert(0, "/opt/trn_rl_repo")
    import concourse.bass as bass

import concourse.mybir as mybir
import concourse.tile as tile
from concourse.bass_utils import run_bass_kernel_spmd
from concourse.masks import make_identity

F32 = mybir.dt.float32
F32R = mybir.dt.float32r
BF16 = mybir.dt.bfloat16

# Problem shape (hardcoded; see module docstring).
BQ, LQ, BD, LD, H, D = 32, 32, 256, 180, 768, 64
NCORES = 8
BD_LOC = BD // NCORES  # 32 docs per core
TD = BD_LOC * LD  # 5760 doc tokens per core
TQ = BQ * LQ  # 1024 query tokens
KT = H // 128  # 6 contraction k-tiles
NB_D = TD // 128  # 45 doc token blocks
NB_Q = TQ // 128  # 8 query token blocks
Q_PER_BLOCK = 128 // LQ  # 4 queries per 128-token block

DOCS_PER_CHUNK = 4  # docs per sim chunk / reduce unit
CHUNK_COLS = DOCS_PER_CHUNK * LD  # 720
N_CHUNKS = BD_LOC // DOCS_PER_CHUNK  # 8
N_UNITS = N_CHUNKS * NB_Q  # 64
DEP_BLOCKS = 4  # token blocks per group / deposit (knob: 4 or 8)

# Tuning knobs (validated against the jax reference):
# Reduce-unit engine assignment cycle:
#   'D' = DVE reduce_max straight from PSUM over the full Ld=180 (1 elem/cyc).
#   'R' = relu-pairmax: max(a,b) = relu(a-b)+b. PE matmuls s_diff = q*(d_even
#         - d_odd) (precomputed dd_ columns) into one PSUM tile and s_odd =
#         q*d_odd (stride-2 rhs view) into another; Act applies Relu
#         (PSUM->SBUF bf16); PE accumulates the relu back onto s_odd
#         (identity matmul, PSUM accumulate); DVE max-reduces the half-width
#         [128, 8, 90] pairmax tile. Splits the reduce across PE/Act/DVE.
#   (GPSIMD tensor ops and dual-PSUM-input DVE ops are rejected by the
#   public neuronxcc - NCC_IXCG966 / NCC_IBVF027 - so no Pool trees.)
def _mk_pattern(**counts):
    # round-robin interleave so consecutive units hit different engines
    total = sum(counts.values())
    out = []
    err = {k: 0.0 for k in counts}
    for i in range(total):
        for k in err:
            err[k] += counts[k] / total
        pick = max(err, key=lambda k: err[k])
        err[pick] -= 1.0
        out.append(pick)
    return "".join(out)


REDUCE_PATTERN = "D" * 16 + "R" * 48
DEPOSIT_MODE = "dve"  # "dve" | "act" | "alt"
SCALE_ENGINE = "dve"  # "act" | "dve"  (who applies the 1/norm scale-copy)
DOC_DMA_TOKENS = 640  # tokens per doc-embedding DMA chunk (9 chunks)
UNITS_PER_GROUP = 4  # reduce units drained per doc group
QRY_DMA_FIRST = False  # load first query chunk before first doc chunk
UNITS_BEFORE_GROUP = False  # drain reduce units before (vs after) each group
DOC_DMA_STAGED = False  # small leading DMA chunks for an earlier ramp
DEFER_OUT_DMA = True  # emit prev pass's result DMA after this pass's input DMAs
SIM_BUFS = 3
STAGE_BUFS = 2
TREE_BUFS = 2


def _build_qmask():
    qmask = np.zeros((128, NB_Q, BQ), dtype=np.float32)
    p = np.arange(128)
    for qb in range(NB_Q):
        qmask[p, qb, qb * Q_PER_BLOCK + p // LQ] = 1.0
    return qmask


class _Pipeline:
    """Per-core kernel body emitter."""

    def __init__(self, tc, pools):
        self.tc = tc
        self.nc = tc.nc
        self.pools = pools

    def process_group(self, src_sb, b0, nsb, ptr_tile):
        """Project+normalize token blocks [b0, b0+nsb) of src_sb
        ([128, KT, ntok] bf16), transpose into ptr_tile[:, j, :]."""
        nc = self.nc
        p = self.pools
        pd = p["ps_pd"].tile([128, DEP_BLOCKS, D], F32, tag="pd")
        for j in range(nsb):
            b = b0 + j
            for k in range(KT):
                nc.tensor.matmul(
                    pd[:, j, :],
                    lhsT=src_sb[:, k, b * 128 : (b + 1) * 128],
                    rhs=self.wt_sb[:, k, :],
                    start=(k == 0),
                    stop=(k == KT - 1),
                )
        ssq = p["small"].tile([128, DEP_BLOCKS, 1], F32, tag="ssq")
        scr = p["scr"].tile([128, D], BF16, tag="scr")
        for j in range(nsb):
            nc.scalar.activation(
                out=scr,
                in_=pd[:, j, :],
                func=mybir.ActivationFunctionType.Square,
                accum_out=ssq[:, j, :],
            )
        nrm = p["small"].tile([128, DEP_BLOCKS, 1], F32, tag="nrm")
        nc.scalar.sqrt(out=nrm[:, 0:nsb, :], in_=ssq[:, 0:nsb, :])
        rn = p["small"].tile([128, DEP_BLOCKS, 1], F32, tag="rn")
        nc.vector.reciprocal(out=rn[:, 0:nsb, :], in_=nrm[:, 0:nsb, :])
        dnrm = p["dn"].tile([128, DEP_BLOCKS, D], BF16, tag="dnrm")
        if SCALE_ENGINE == "act":
            for j in range(nsb):
                nc.scalar.activation(
                    out=dnrm[:, j, :],
                    in_=pd[:, j, :],
                    func=mybir.ActivationFunctionType.Copy,
                    scale=rn[:, j, :],
                )
        else:
            # One DVE tensor_tensor over the whole group; rn broadcast
            # (stride-0) over the 64-wide projection dim.
            rn_b, pd_b = bass.broadcast_tensor_aps(
                rn[:, 0:nsb, :], pd[:, 0:nsb, :]
            )
            nc.vector.tensor_tensor(
                out=dnrm[:, 0:nsb, :],
                in0=pd_b,
                in1=rn_b,
                op=mybir.AluOpType.mult,
            )
        for j in range(nsb):
            nc.tensor.transpose(
                ptr_tile[:, j, :], dnrm[:, j, :], self.ident
            )

    def emit_group(self, src_sb, b0, ndep, out_t, dd=None):
        """Project/normalize/transpose/deposit one group of `ndep` blocks.
        For docs (dd not None) also build the pair-diff columns dd_."""
        nc = self.nc
        p = self.pools
        ptr = p["ps_tr"].tile([64, DEP_BLOCKS, 128], BF16, tag="ptr")
        self.process_group(src_sb, b0, ndep, ptr)
        cols = ndep * 128
        view = ptr.rearrange("p j t -> p (j t)")[:, 0:cols]
        use_dve = DEPOSIT_MODE == "dve" or (
            DEPOSIT_MODE == "alt" and (b0 // DEP_BLOCKS) % 2 == 0
        )
        if use_dve:
            nc.vector.tensor_copy(
                out=out_t[:, b0 * 128 : b0 * 128 + cols], in_=view
            )
        else:
            nc.scalar.copy(
                out=out_t[:, b0 * 128 : b0 * 128 + cols], in_=view
            )
        if dd is not None:
            # dd[:, t] = d_even - d_odd for each within-doc token pair (Ld is
            # even, deposits are 512-aligned, so global parity == pair parity).
            seg = out_t[:, b0 * 128 : b0 * 128 + cols].rearrange(
                "p (t two) -> p two t", two=2
            )
            nc.vector.tensor_tensor(
                out=dd[:, b0 * 64 : b0 * 64 + cols // 2],
                in0=seg[:, 0, :],
                in1=seg[:, 1, :],
                op=mybir.AluOpType.subtract,
            )

    def emit_unit(self, c, qb):
        """Sim + max-reduce for chunk c (4 docs) x one query block.
        Single-bank PSUM tiles so several units pipeline concurrently."""
        nc = self.nc
        p = self.pools
        nd = DOCS_PER_CHUNK
        unit = self.unit_idx
        self.unit_idx += 1
        eng = REDUCE_PATTERN[unit % len(REDUCE_PATTERN)]
        out_view = self.maxsim[:, qb, c * nd : (c + 1) * nd]
        lhs = self.qt_[:, qb * 128 : (qb + 1) * 128]
        col0 = c * CHUNK_COLS
        if eng == "D":
            # two 2-doc fills + direct reduces (a 720-col chunk spans banks)
            for h, tag in enumerate(("simo", "simd")):
                ps = p["ps_s"].tile([128, 512], F32, tag=tag)
                nc.tensor.matmul(
                    ps[:, 0:360],
                    lhsT=lhs,
                    rhs=self.dt_[:, col0 + h * 360 : col0 + (h + 1) * 360],
                    start=True,
                    stop=True,
                )
                nc.vector.reduce_max(
                    out=out_view[:, h * 2 : h * 2 + 2],
                    in_=ps[:, 0:360].rearrange("p (d l) -> p d l", l=LD),
                    axis=mybir.AxisListType.X,
                )
            return
        # "R": relu-pairmax over the chunk's 360 token pairs.
        ps_o = p["ps_s"].tile([128, 512], F32, tag="simo")
        ps_d = p["ps_s"].tile([128, 512], F32, tag="simd")
        odd = self.dt_[:, col0 : col0 + CHUNK_COLS].rearrange(
            "p (t two) -> p two t", two=2
        )[:, 1, :]
        dcol = col0 // 2
        nc.tensor.matmul(
            ps_o[:, 0:360], lhsT=lhs, rhs=odd, start=True, stop=False
        )
        nc.tensor.matmul(
            ps_d[:, 0:360],
            lhsT=lhs,
            rhs=self.dd_[:, dcol : dcol + 360],
            start=True,
            stop=True,
        )
        stage = p["stage"].tile([128, 360], BF16, tag="stage")
        nc.scalar.activation(
            out=stage,
            in_=ps_d[:, 0:360],
            func=mybir.ActivationFunctionType.Relu,
        )
        # PE accumulates relu(s_diff) onto s_odd (identity matmul, PSUM acc).
        nc.tensor.matmul(
            ps_o[:, 0:360], lhsT=self.ident, rhs=stage, start=False, stop=True
        )
        nc.vector.reduce_max(
            out=out_view,
            in_=ps_o[:, 0:360].rearrange("p (d l) -> p d l", l=LD // 2),
            axis=mybir.AxisListType.X,
        )

    def emit_scores(self, scores_out):
        nc = self.nc
        p = self.pools
        scores_ps = p["ps_pd"].tile([128, DEP_BLOCKS, D], F32, tag="pd")
        sp = scores_ps.rearrange("p j d -> p (j d)")[0:BQ, 0:BD_LOC]
        for qb in range(NB_Q):
            nc.tensor.matmul(
                sp,
                lhsT=self.qmask_sb[:, qb, :],
                rhs=self.maxsim[:, qb, :],
                start=(qb == 0),
                stop=(qb == NB_Q - 1),
            )
        scores_sb = p["small"].tile([BQ, BD_LOC], F32, tag="scores")
        nc.vector.tensor_copy(out=scores_sb, in_=sp)
        return scores_sb


def _kernel_body(tc, doct, qryt, wt, qmask, scores_out, repeat=1):
    nc = tc.nc
    with (
        tc.tile_pool(name="const", bufs=1) as const,
        tc.tile_pool(name="data", bufs=1) as data,
        tc.tile_pool(name="dn", bufs=2) as dn,
        tc.tile_pool(name="scr", bufs=2) as scr,
        tc.tile_pool(name="small", bufs=4) as small,
        tc.tile_pool(name="stage", bufs=STAGE_BUFS) as stage,
        tc.tile_pool(name="ps_pd", bufs=1, space="PSUM") as ps_pd,
        tc.tile_pool(name="ps_tr", bufs=1, space="PSUM") as ps_tr,
        tc.tile_pool(name="ps_s", bufs=SIM_BUFS, space="PSUM") as ps_s,
    ):
        ident_f = const.tile([128, 128], F32)
        make_identity(nc, ident_f)
        ident = const.tile([128, 128], BF16, name="ident_bf16")
        nc.vector.tensor_copy(out=ident, in_=ident_f)

        pools = {
            "dn": dn,
            "scr": scr,
            "small": small,
            "stage": stage,
            "ps_pd": ps_pd,
            "ps_tr": ps_tr,
            "ps_s": ps_s,
        }
        pipe = _Pipeline(tc, pools)
        pipe.ident = ident

        def _one_pass(slot=0):
            # Two SBUF slots for the big doc-embedding buffer (and the tiny
            # constants) let pass k+1's input DMA prefetch underneath pass
            # k's compute tail when the repeat body emits two passes.
            pipe.unit_idx = 0
            wt_sb = data.tile([128, KT, D], BF16, tag=f"wt{slot}")
            qmask_sb = data.tile([128, NB_Q, BQ], F32, tag=f"qmask{slot}")
            qryt_sb = data.tile([128, KT, TQ], BF16, tag="qryt")
            doct_sb = data.tile([128, KT, TD], BF16, tag=f"doct{slot}")
            qt_ = data.tile([64, TQ], BF16, tag=f"qt{slot}")
            dt_ = data.tile([64, TD], BF16, tag="dt")
            dd_ = data.tile([64, TD // 2], BF16, tag="dd")
            maxsim = data.tile([128, NB_Q, BD_LOC], F32, tag=f"maxsim{slot}")
            pipe.wt_sb = wt_sb
            pipe.qmask_sb = qmask_sb
            pipe.qt_ = qt_
            pipe.dt_ = dt_
            pipe.dd_ = dd_
            pipe.maxsim = maxsim

            # --- streaming driver -------------------------------------------
            # The DMA engines are effectively a serial resource and the five
            # engine queues are in-order, so emission order IS the schedule.
            # Interleave: doc-token DMA chunks -> projection groups (as their
            # tokens land) -> deposits -> sim/reduce units (as their doc
            # columns land), with the query phase slotted behind its own DMAs.
            if DOC_DMA_STAGED:
                sched = [256, 384]
                while sum(sched) < TD - DOC_DMA_TOKENS:
                    sched.append(DOC_DMA_TOKENS)
                sched.append(TD - sum(sched))
            else:
                sched = [DOC_DMA_TOKENS] * (TD // DOC_DMA_TOKENS)
            doc_edges = [0]
            for s in sched:
                doc_edges.append(doc_edges[-1] + s)
            qry_chunk = TQ // 2
            state = {"doc_dma": 0, "qry_dma": 0}

            def dma_doc_chunk():
                i = state["doc_dma"]
                lo, hi = doc_edges[i], doc_edges[i + 1]
                nc.sync.dma_start(
                    out=doct_sb[:, :, lo:hi],
                    in_=doct[:, lo:hi].rearrange("(k p) t -> p k t", p=128),
                )
                state["doc_dma"] += 1

            def dma_qry_chunk():
                i = state["qry_dma"]
                lo, hi = i * qry_chunk, (i + 1) * qry_chunk
                nc.sync.dma_start(
                    out=qryt_sb[:, :, lo:hi],
                    in_=qryt[:, lo:hi].rearrange("(k p) t -> p k t", p=128),
                )
                state["qry_dma"] += 1

            def need_doc_tokens(tok):
                while (
                    state["doc_dma"] < len(sched)
                    and doc_edges[state["doc_dma"]] < min(tok, TD)
                ):
                    dma_doc_chunk()

            # Upfront: weights, first doc chunk, first query chunk.
            nc.sync.dma_start(
                out=wt_sb, in_=wt[:, :].rearrange("(k p) d -> p k d", p=128)
            )
            if QRY_DMA_FIRST:
                dma_qry_chunk()
                dma_doc_chunk()
            else:
                dma_doc_chunk()
                dma_qry_chunk()
            nc.sync.dma_start(out=qmask_sb, in_=qmask[:, :, :])
            if DEFER_OUT_DMA:
                # Emit the remaining input DMAs now and the PREVIOUS pass's
                # scores-out DMA after them: the scores DMA waits on the
                # previous pass's tail, and the SP queue is in-order - with
                # it emitted first it head-of-line-blocks this pass's entire
                # input prefetch.
                dma_qry_chunk()
                need_doc_tokens(TD)
                if prev_out[0] is not None:
                    nc.sync.dma_start(out=scores_out[:, :], in_=prev_out[0])
                    prev_out[0] = None

            def doc_group(g):
                b0 = g * DEP_BLOCKS
                ndep = min(DEP_BLOCKS, NB_D - b0)
                need_doc_tokens((b0 + ndep) * 128)
                pipe.emit_group(doct_sb, b0, ndep, dt_, dd=dd_)
                return (b0 + ndep) * 128

            def qry_group(g):
                b0 = g * DEP_BLOCKS
                ndep = min(DEP_BLOCKS, NB_Q - b0)
                if ndep > 0:
                    pipe.emit_group(qryt_sb, b0, ndep, qt_)

            n_doc_groups = (NB_D + DEP_BLOCKS - 1) // DEP_BLOCKS  # 12
            units = []  # ready (chunk, qb) backlog
            state["next_chunk"] = 0
            state["qt_done"] = False

            def update_units(cols_done):
                while (
                    state["next_chunk"] < N_CHUNKS
                    and (state["next_chunk"] + 1) * CHUNK_COLS <= cols_done
                ):
                    units.extend(
                        (state["next_chunk"], qb) for qb in range(NB_Q)
                    )
                    state["next_chunk"] += 1

            def drain_units(k):
                while units and k > 0:
                    cc, qb = units.pop(0)
                    pipe.emit_unit(cc, qb)
                    k -= 1

            # Schedule: doc groups stream with their DMAs; the query phase
            # rides behind its two DMA chunks early on (PE idles on doc DMA
            # anyway); units drain as their doc columns deposit.
            cols_done = 0
            for g in range(n_doc_groups):
                if state["qt_done"] and UNITS_BEFORE_GROUP:
                    update_units(cols_done)
                    drain_units(UNITS_PER_GROUP)
                cols_done = doc_group(g)
                if g == 0:
                    qry_group(0)
                    if state["qry_dma"] < 2:
                        dma_qry_chunk()
                elif g == 1:
                    qry_group(1)
                    state["qt_done"] = True
                if state["qt_done"] and not UNITS_BEFORE_GROUP:
                    update_units(cols_done)
                    drain_units(UNITS_PER_GROUP)
            update_units(cols_done)
            drain_units(len(units))

            scores_sb = pipe.emit_scores(scores_out)
            if DEFER_OUT_DMA:
                prev_out[0] = scores_sb
            else:
                nc.sync.dma_start(out=scores_out[:, :], in_=scores_sb)

        prev_out = [None]

        def _flush_out():
            if prev_out[0] is not None:
                nc.sync.dma_start(out=scores_out[:, :], in_=prev_out[0])
                prev_out[0] = None

        if repeat == 1:
            _one_pass(0)
            _flush_out()
        else:
            n2, rem = divmod(repeat, 2)
            with tc.For_i(0, n2, 1):
                _one_pass(0)
                _one_pass(1)
                _flush_out()
            for _ in range(rem):
                _one_pass(0)
                _flush_out()


def split_multi_waits(nc, max_waits=1):
    """The public neuronxcc walrus only encodes one inline sync-wait per
    instruction; Tile's scheduler attaches several. Split the excess into
    preceding same-engine nop-waits (engine queues execute in order, so the
    semantics are identical)."""
    for f in nc.m.functions:
        for blk in f.blocks:
            new_insts = []
            for inst in blk.instructions:
                si = inst.sync_info
                if si is not None and len(si.on_wait) > max_waits:
                    waits = list(si.on_wait)
                    for w in waits[:-max_waits]:
                        new_insts.append(
                            mybir.InstNoOp(
                                name=nc.get_next_instruction_name(),
                                ins=[],
                                outs=[],
                                engine=inst.engine,
                                sync_info=mybir.SyncInfo(on_wait=[w], on_update=[]),
                            )
                        )
                    inst.sync_info = mybir.SyncInfo(
                        on_wait=waits[-max_waits:], on_update=list(si.on_update)
                    )
                new_insts.append(inst)
            blk.instructions = new_insts
    return nc


def build_bass(repeat=1, split_waits=True, **knobs):
    global REDUCE_PATTERN, DEPOSIT_MODE, DOC_DMA_TOKENS, SIM_BUFS, \
        STAGE_BUFS, SCALE_ENGINE, TREE_BUFS, UNITS_PER_GROUP, \
        QRY_DMA_FIRST, UNITS_BEFORE_GROUP, DOC_DMA_STAGED, DEP_BLOCKS, \
        DEFER_OUT_DMA
    old = (REDUCE_PATTERN, DEPOSIT_MODE, DOC_DMA_TOKENS, SIM_BUFS,
           STAGE_BUFS, SCALE_ENGINE, TREE_BUFS, UNITS_PER_GROUP,
           QRY_DMA_FIRST, UNITS_BEFORE_GROUP, DOC_DMA_STAGED, DEP_BLOCKS,
           DEFER_OUT_DMA)
    REDUCE_PATTERN = knobs.get("reduce_pattern", REDUCE_PATTERN)
    DEPOSIT_MODE = knobs.get("deposit_mode", DEPOSIT_MODE)
    DOC_DMA_TOKENS = knobs.get("doc_dma_tokens", DOC_DMA_TOKENS)
    UNITS_PER_GROUP = knobs.get("units_per_group", UNITS_PER_GROUP)
    QRY_DMA_FIRST = knobs.get("qry_dma_first", QRY_DMA_FIRST)
    UNITS_BEFORE_GROUP = knobs.get("units_before_group", UNITS_BEFORE_GROUP)
    DOC_DMA_STAGED = knobs.get("doc_dma_staged", DOC_DMA_STAGED)
    DEFER_OUT_DMA = knobs.get("defer_out_dma", DEFER_OUT_DMA)
    DEP_BLOCKS = knobs.get("dep_blocks", DEP_BLOCKS)
    SIM_BUFS = knobs.get("sim_bufs", SIM_BUFS)
    STAGE_BUFS = knobs.get("stage_bufs", STAGE_BUFS)
    SCALE_ENGINE = knobs.get("scale_engine", SCALE_ENGINE)
    TREE_BUFS = knobs.get("tree_bufs", TREE_BUFS)
    try:
        return _build_bass_inner(repeat, split_waits)
    finally:
        (REDUCE_PATTERN, DEPOSIT_MODE, DOC_DMA_TOKENS, SIM_BUFS,
         STAGE_BUFS, SCALE_ENGINE, TREE_BUFS, UNITS_PER_GROUP,
         QRY_DMA_FIRST, UNITS_BEFORE_GROUP, DOC_DMA_STAGED,
         DEP_BLOCKS, DEFER_OUT_DMA) = old


def _build_bass_inner(repeat, split_waits=True):
    nc = bass.Bass()
    doct = nc.dram_tensor("doct", [H, TD], BF16, kind="ExternalInput")
    qryt = nc.dram_tensor("qryt", [H, TQ], BF16, kind="ExternalInput")
    wt = nc.dram_tensor("wt", [H, D], BF16, kind="ExternalInput")
    qmask = nc.dram_tensor("qmask", [128, NB_Q, BQ], F32, kind="ExternalInput")
    scores_out = nc.dram_tensor("scores", [BQ, BD_LOC], F32, kind="ExternalOutput")
    with tile.TileContext(nc) as tc:
        _kernel_body(tc, doct, qryt, wt, qmask, scores_out, repeat=repeat)
    if split_waits:
        split_multi_waits(nc)
    return nc


_NC_CACHE = None


def _get_nc():
    global _NC_CACHE
    if _NC_CACHE is None:
        _NC_CACHE = build_bass()
    return _NC_CACHE


def _make_in_maps(qry_emb, doc_emb, W):
    import ml_dtypes

    bf = ml_dtypes.bfloat16
    wt = np.ascontiguousarray(W.T.astype(bf))  # [768, 64]
    qryt = np.ascontiguousarray(
        qry_emb.reshape(TQ, H).T.astype(bf)
    )  # [768, 1024]
    qmask = _build_qmask()
    in_maps = []
    for c in range(NCORES):
        doct = np.ascontiguousarray(
            doc_emb[c * BD_LOC : (c + 1) * BD_LOC].reshape(TD, H).T.astype(bf)
        )
        in_maps.append({"doct": doct, "qryt": qryt, "wt": wt, "qmask": qmask})
    return in_maps


def _finish_loss(score_blocks, group_size):
    scores = np.concatenate(score_blocks, axis=1).astype(np.float64)  # [32, 256]
    labels = np.arange(BQ) * int(group_size)
    m = scores.max(axis=1, keepdims=True)
    lse = m[:, 0] + np.log(np.exp(scores - m).sum(axis=1))
    loss = np.mean(lse - scores[np.arange(BQ), labels])
    return np.float32(loss)


def kernel(qry_emb, doc_emb, W, group_size, _trace=False):
    nc = _get_nc()
    in_maps = _make_in_maps(np.asarray(qry_emb), np.asarray(doc_emb), np.asarray(W))
    res = run_bass_kernel_spmd(nc, in_maps, list(range(NCORES)), trace=_trace)
    blocks = [res.results[c]["scores"] for c in range(NCORES)]
    loss = _finish_loss(blocks, group_size)
    if _trace:
        return loss, res
    return loss
